# revision 29
# baseline (speedup 1.0000x reference)
"""BandSplit (gather -> per-band MLP -> scatter-add OLA -> /ola) on 8 TRN2 cores.

Strategy
--------
The whole reference computation is linear in x: fold everything into one
block-banded matrix A of shape (C*F, C*F) so that per (b, t) token
out = A^T vec(x) + const (const == 0 here; added on host regardless).
Data-parallel over the 4096 (b, t) tokens across 8 cores, 512 tokens each,
no cross-core communication.

Key structure:
 - Contraction chunks are 64 f-rows x 2 channels interleaved on the 128
   partitions (p = 2*fl + ci).  Halving the row span shrinks each chunk's
   output window (band support), cutting total matmul columns from ~27.8k
   (128-row, per-channel chunks) to ~19.1k cycles, and both input channels
   ride one matmul.  The lone f=1024 row (and the bias constant) is added
   on host in exact f32, so the device stream is 16 uniform chunks.
 - The packed band matrix `ab` ships as fp8 e3m4 scaled by 2^6 with the
   2^-6 folded into the host-side bf16 cast of x, so PSUM holds true-scale
   outputs and drains are plain f32->f16 copies.  Mixed bf16(x) x fp8(A)
   matmuls run at the full 1 col/cycle rate.
 - Every dma_start blocks its issuing sequencer for ~600 ns (descriptor
   generation for 128 partition lines) and a queue's DMAs move FIFO with
   ~0.5-1 us completion latency each, so DMAs are few and fat (xs in
   chunk-pairs, ab in 2 slabs) and spread over three parallel queues (SP +
   ACT HWDGE rings, GPSIMD SWDGE); y stores bank-major so each PSUM bank
   ships as one contiguous DMA.
 - The stream runs REVERSED (j15 -> j0): wide chunks demand ~2x fewer
   bytes per PE cycle, so consuming them first matches the delivery ramp
   while the narrow, bandwidth-hungry chunks run last from resident data.
 - A 32-matmul junk burst bridges PE-boot -> first-data AND covers a full
   free-running HAM window (3.4 us) so the clock latches 2.4 GHz before
   the real stream; mid-stream stalls under ~3.4 us then cannot de-latch.
 - PSUM bank lifetimes are 2-colored per token chunk (8 banks total);
   each bank drains (DVE/ACT alternating) right after its last chunk and
   stores immediately, with the final bank stored per-tch to overlap the
   postamble edge.
"""

import numpy as np

_P = 128
_C = 2
_F = 1025
_R = 64                     # f-rows per contraction chunk (x2 ci = 128)
_NJ = 16                    # chunks j cover f in [64j, 64j+64); f=1024 is added on host
_TCH = 4                    # token chunks (of 128) per core
_TCORE = _TCH * _P          # 512 tokens per core
_PS_W = _C * _F             # 2050 output columns (col = 2*fo + co)
# PSUM banks: 512-col pieces; the last 2 cols ride a separate 2-col bank but
# share bank 3's slot in the output layout (width 514).
_BANKS = [(0, 512), (512, 1024), (1024, 1536), (1536, 2048), (2048, 2050)]
_LAYW = [512, 512, 512, 514]          # output-layout widths (bank4 merged into 3)
_LAYB = [0, 2048, 4096, 6144]         # block base: b*4*width
_YW = 6144 + 4 * 514                  # 8200
_SCALE_BITS = 6             # ab * 2^6 in fp8; x * 2^-6 in bf16


def _fold_matrix(pre_w, pre_b, post_w, post_b, idx, melw, mask, ola_window):
    """Fold the full reference computation into (A, const).

    A: (C, F, C, F) with out[co, fo] = sum_{ci, fi} x[ci, fi] * A[ci, fi, co, fo]
    const: (C, F) additive constant from the biases.
    """
    K, W = idx.shape
    C = _C
    F = ola_window.shape[0]

    pre_w = np.asarray(pre_w, np.float64)
    post_w = np.asarray(post_w, np.float64)
    pre_b = np.asarray(pre_b, np.float64)
    post_b = np.asarray(post_b, np.float64)
    wts = (np.asarray(melw, np.float64) * np.asarray(mask, np.float64))
    msk = np.asarray(mask, np.float64)
    idx = np.asarray(idx)

    M = np.einsum('kio,koj->kij', pre_w, post_w).reshape(K, W, C, W, C)
    vals = M * wts[:, :, None, None, None] * msk[:, None, None, :, None]

    fin = idx[:, :, None, None, None].astype(np.int64)
    fout = idx[:, None, None, :, None].astype(np.int64)
    cin = np.arange(C)[None, None, :, None, None]
    cout = np.arange(C)[None, None, None, None, :]
    flat = ((cin * F + fin) * C + cout) * F + fout
    A = np.bincount(
        np.broadcast_to(flat, vals.shape).ravel(), weights=vals.ravel(),
        minlength=C * F * C * F,
    ).reshape(C, F, C, F)
    A /= ola_window[None, None, None, :]

    bv = (np.einsum('ko,koj->kj', pre_b, post_w) + post_b).reshape(K, W, C)
    bv = bv * msk[:, :, None]
    cflat = (np.arange(C)[None, None, :] * F + idx[:, :, None]).astype(np.int64)
    const = np.bincount(
        np.broadcast_to(cflat, bv.shape).ravel(), weights=bv.ravel(),
        minlength=C * F,
    ).reshape(C, F)
    const /= ola_window[None, :]
    return A, const


def _plan(A, order=None):
    """Windows, packed offsets, bank touch lists and PSUM slot colors.

    `order` is the stream emission order over chunks (default ascending).
    Bank first/last are STREAM POSITIONS (indices into `order`).
    """
    if order is None:
        order = list(range(_NJ))
    wins = []                   # j -> (lo, hi) in fo units
    for j in range(_NJ):
        f0, f1 = j * _R, min((j + 1) * _R, _F)
        blk = A[:, f0:f1, :, :]
        cols = (blk != 0).any(axis=(0, 1, 2))
        nzc = np.nonzero(cols)[0]
        assert len(nzc) > 0
        wins.append((int(nzc[0]), int(nzc[-1]) + 1))
    covered = np.zeros(_F, bool)
    for lo, hi in wins:
        covered[lo:hi] = True
    assert covered.all(), "window coverage hole"

    offs = {}
    tw = 0
    for j in range(_NJ):
        offs[j] = tw
        tw += (2 * (wins[j][1] - wins[j][0]) + 15) // 16 * 16

    touches = {}                # b -> ordered [(pos, s, e)] in stream order
    for pos, j in enumerate(order):
        lo2, hi2 = 2 * wins[j][0], 2 * wins[j][1]
        for b, (bs, be) in enumerate(_BANKS):
            s, e = max(lo2, bs), min(hi2, be)
            if s < e:
                touches.setdefault(b, []).append((pos, s, e))
    first_p = {b: t[0][0] for b, t in touches.items()}
    last_p = {b: t[-1][0] for b, t in touches.items()}

    colors = {}
    for b in sorted(touches):
        used = {colors[o] for o in colors
                if not (last_p[o] < first_p[b] or last_p[b] < first_p[o])}
        free = [c for c in "AB" if c not in used]
        assert free, f"PSUM slot coloring needs >2 colors at bank {b}"
        colors[b] = free[0]
    return wins, offs, tw, touches, first_p, last_p, colors, order


def _olay(b, tch):
    """Output-layout (base, width) for PSUM bank b, token chunk tch."""
    lb = min(b, 3)
    base = _LAYB[lb] + tch * _LAYW[lb]
    if b == 4:
        base += 512
    return base


_PROGRAM_CACHE = {}


def _build_program(wins, offs, TW, touches, first_p, last_p, colors, order,
                   n_cores):
    import concourse.tile as tile
    import concourse.mybir as mybir
    from concourse import bacc

    f32 = mybir.dt.float32
    bf16 = mybir.dt.bfloat16
    f16 = mybir.dt.float16
    f8e3 = mybir.dt.float8e3
    P = _P
    XCOLS = _NJ * _TCORE         # 8192 cols: j*512 + tch*128 + tok

    nc = bacc.Bacc("TRN2", target_bir_lowering=False, debug=False,
                   num_devices=n_cores)
    xs = nc.dram_tensor("xs", [P, XCOLS], bf16, kind="ExternalInput")
    ab = nc.dram_tensor("ab", [P, TW], f8e3, kind="ExternalInput")
    # y: bank-major f16; block b at _LAYB[b], piece (b, tch) at _olay(b, tch)
    y = nc.dram_tensor("y", [P, _YW], f16, kind="ExternalOutput")

    with tile.TileContext(nc) as tc:
        with (
            tc.tile_pool(name="apool", bufs=1) as apool,
            tc.tile_pool(name="xpool", bufs=1) as xpool,
            tc.tile_pool(name="opool", bufs=1) as opool,
            tc.tile_pool(name="jpool", bufs=1) as jpool,
            tc.tile_pool(name="pspool", bufs=1, space="PSUM") as pspool,
        ):
            abig = apool.tile([P, TW], f8e3, name="abig")
            xbig = xpool.tile([P, XCOLS], bf16, name="xbig")
            junk = jpool.tile([P, P], bf16, name="junk")
            ot = opool.tile([P, _YW], f16, name="ot")

            nc.vector.memset(junk[:], 0.0)
            # ~1.7 us dummy on the Q7 delays the GPSIMD queue's first DMA so
            # the two critical first pieces (SP xs pair + ACT ab slab) get
            # the full HBM rate instead of a 3-way split.
            dummy = jpool.tile([P, 2048], f32, name="dummy")
            nc.gpsimd.memset(dummy[:], 0.0)

            # Loads: both HWDGE rings issue in parallel (~600 ns per
            # dma_start on the issuing sequencer), and each ring moves its
            # queue in FIFO order.  The first pieces are small so the j0/j1
            # completion semaphores fire early; per-ring order tracks the
            # stream's consumption order and bytes are balanced per ring.
            def xs_load(eng, j0, j1):
                eng.dma_start(xbig[:, j0 * _TCORE:j1 * _TCORE],
                              xs[:, j0 * _TCORE:j1 * _TCORE])

            def ab_load(eng, j0, j1):
                o0 = offs[j0]
                o1 = offs[j1] if j1 < _NJ else TW
                eng.dma_start(abig[:, o0:o1], ab[:, o0:o1])

            # Reversed stream (j15 -> j0): wide chunks first, whose
            # bytes-per-PE-cycle demand is ~2x lower, so early delivery
            # keeps up; narrow chunks run last when all data is resident.
            # Three parallel queues: SP + ACT (HWDGE) + GPSIMD (SWDGE),
            # each queue's FIFO matching the stream's consumption order.
            xs_load(nc.sync, 14, 16)     # j14, j15 (first on SP)
            ab_load(nc.scalar, 10, 16)   # ab tail slab (first on ACT)
            xs_load(nc.gpsimd, 12, 14)   # j12, j13 (first on GPS)
            xs_load(nc.sync, 10, 12)     # j10, j11
            ab_load(nc.scalar, 0, 10)    # ab head slab
            xs_load(nc.gpsimd, 8, 10)    # j8, j9
            xs_load(nc.sync, 6, 8)       # j6, j7
            xs_load(nc.scalar, 4, 6)     # j4, j5
            xs_load(nc.sync, 2, 4)       # j2, j3
            xs_load(nc.scalar, 0, 2)     # j0, j1

            # Warmup burst: bridge PE-boot -> first data, keep the HAM
            # continuous-activity window alive into the real stream.  The
            # warm tile shares the first-touched bank's PSUM slot.
            lo2_0 = 2 * wins[order[0]][0]
            first_bank = min(b for b, (bs, be) in enumerate(_BANKS)
                             if max(lo2_0, bs) < min(2 * wins[order[0]][1], be))
            warm = pspool.tile([P, 512], f32, tag=f"{colors[first_bank]}0",
                               name="warm")
            for _ in range(32):
                nc.tensor.matmul(warm[:, :P], junk[:], junk[:],
                                 start=True, stop=True)

            cur = {}                    # (tch, b) -> PSUM tile
            drain_rr = [0]

            def drain(tch, b):
                base = _olay(b, tch)
                w = _BANKS[b][1] - _BANKS[b][0]
                t = cur.pop((tch, b))
                if drain_rr[0] % 2 == 0:
                    nc.vector.tensor_copy(ot[:, base:base + w], t[:])
                else:
                    nc.scalar.copy(ot[:, base:base + w], t[:])
                drain_rr[0] += 1

            def emit_mms(pos, tch):
                j = order[pos]
                lo2 = 2 * wins[j][0]
                lhsT = xbig[:, j * _TCORE + tch * P:j * _TCORE + (tch + 1) * P]
                o = offs[j]
                for b, (bs, be) in enumerate(_BANKS):
                    s, e = max(lo2, bs), min(2 * wins[j][1], be)
                    if s >= e:
                        continue
                    if (tch, b) not in cur:
                        cur[(tch, b)] = pspool.tile(
                            [P, be - bs], f32, tag=f"{colors[b]}{tch}",
                            name=f"bk{b}_{tch}")
                    nc.tensor.matmul(
                        cur[(tch, b)][:, s - bs:e - bs],
                        lhsT, abig[:, o + s - lo2:o + e - lo2],
                        start=(touches[b][0][0] == pos),
                        stop=(touches[b][-1][0] == pos),
                    )

            # output-layout groups: bank 4 shares bank 3's 514-wide block;
            # a group stores once ALL member banks have drained
            groups = {0: [0], 1: [1], 2: [2], 3: [3, 4]}
            gstore_pos = {g: max(last_p[b] for b in m if b in touches)
                          for g, m in groups.items()}
            last_pos = len(order) - 1
            store_rr = [0]

            # main stream in `order`; emission matches per-queue arrival
            for pos in range(last_pos):
                for tch in range(_TCH):
                    emit_mms(pos, tch)
                for b in sorted(touches):
                    if last_p[b] == pos:
                        for tch in range(_TCH):
                            drain(tch, b)
                for g in groups:
                    if gstore_pos[g] == pos:
                        base, w4 = _LAYB[g], 4 * _LAYW[g]
                        eng = nc.sync if store_rr[0] % 2 == 0 else nc.scalar
                        store_rr[0] += 1
                        eng.dma_start(y[:, base:base + w4],
                                      ot[:, base:base + w4])

            # tail: per token chunk, the last chunk + drains + per-tch
            # quarter stores, alternating rings
            tail_banks = sorted(b for b in touches if last_p[b] == last_pos)
            tail_groups = sorted(g for g in groups if gstore_pos[g] == last_pos)
            for tch in range(_TCH):
                emit_mms(last_pos, tch)
                for b in tail_banks:
                    drain(tch, b)
                for g in tail_groups:
                    base, w = _LAYB[g], _LAYW[g]
                    eng = nc.sync if (tch + g) % 2 == 0 else nc.scalar
                    eng.dma_start(y[:, base + tch * w:base + (tch + 1) * w],
                                  ot[:, base + tch * w:base + (tch + 1) * w])

    nc.compile()
    return nc


def kernel(**inputs):
    import ml_dtypes

    x = np.ascontiguousarray(np.asarray(inputs["x"], np.float32))
    B, C, T, F = x.shape
    assert (B, C, F) == (4, 2, 1025), (B, C, F)
    N_CORES = 8
    TS = T // N_CORES

    A, const = _fold_matrix(
        inputs["pre_w"], inputs["pre_b"], inputs["post_w"], inputs["post_b"],
        inputs["idx"], inputs["melw"], inputs["mask"], inputs["ola_window"],
    )
    A = A.astype(np.float32)
    order = list(range(_NJ - 1, -1, -1))      # reversed stream: j15 -> j0
    wins, offs, TW, touches, first_p, last_p, colors, order = _plan(A, order)

    # packed fp8 band tensor, scaled by 2^6
    ab = np.zeros((_P, TW), ml_dtypes.float8_e3m4)
    for j in range(_NJ):
        lo, hi = wins[j]
        f0, f1 = j * _R, min((j + 1) * _R, _F)
        blk = A[:, f0:f1, :, lo:hi]                   # (ci, fl, co, w)
        q = np.clip(blk * np.float32(2.0 ** _SCALE_BITS), -15.5, 15.5)
        t = q.transpose(1, 0, 3, 2).reshape(2 * (f1 - f0), 2 * (hi - lo))
        ab[0:2 * (f1 - f0), offs[j]:offs[j] + 2 * (hi - lo)] = \
            t.astype(ml_dtypes.float8_e3m4)

    key = (TW, tuple(wins), tuple(order), N_CORES)
    if key not in _PROGRAM_CACHE:
        _PROGRAM_CACHE[key] = _build_program(
            wins, offs, TW, touches, first_p, last_p, colors, order, N_CORES)
    nc = _PROGRAM_CACHE[key]

    # host-side cast: bf16(x * 2^-6); the 2^6 lives in ab
    xq = (x * np.float32(2.0 ** -_SCALE_BITS)).astype(ml_dtypes.bfloat16)

    in_maps = []
    for m in range(N_CORES):
        sl = xq[:, :, m * TS:(m + 1) * TS, :1024]     # (tch, ci, t, 1024)
        sl = sl.reshape(_TCH, _C, TS, _NJ, _R)        # (tch, ci, t, j, fl)
        xs_m = np.ascontiguousarray(
            sl.transpose(4, 1, 3, 0, 2)               # (fl, ci, j, tch, t)
        ).reshape(_P, _NJ * _TCORE)
        in_maps.append({"xs": xs_m, "ab": ab})

    try:
        import antenv.axon_hooks  # noqa: F401
    except ImportError:
        import sys
        import types
        import antenv
        stub = types.ModuleType("antenv.axon_hooks")
        stub.get_axon_ntff_profile_hook = lambda: None
        stub.set_axon_ntff_profile_hook = lambda h: None
        sys.modules["antenv.axon_hooks"] = stub
        antenv.axon_hooks = stub

    from concourse.bass_utils import run_bass_kernel_spmd
    res = run_bass_kernel_spmd(nc, in_maps, core_ids=list(range(N_CORES)))
    globals()["_LAST_RESULT"] = res

    out = np.empty((B, C, T, F), np.float32)
    for m in range(N_CORES):
        ym = res.results[m]["y"].astype(np.float32)   # (128, 8200) bank-major
        cols = np.empty((_P, _TCH, _PS_W), np.float32)
        for b in range(4):
            w = _LAYW[b]
            blk = ym[:, _LAYB[b]:_LAYB[b] + 4 * w].reshape(_P, _TCH, w)
            cols[:, :, 512 * b:512 * b + w] = blk
        ym4 = cols.reshape(_P, _TCH, F, C).transpose(1, 3, 0, 2)
        out[:, :, m * TS:(m + 1) * TS, :] = ym4
    # f = 1024 input row: computed on host, unquantized f32 (exact)
    row = A[:, 1024, :, :]                            # (ci, co, fo)
    nzc = np.nonzero((row != 0).any(axis=(0, 1)))[0]
    if len(nzc):
        lo, hi = int(nzc[0]), int(nzc[-1]) + 1
        out[:, :, :, lo:hi] += np.einsum(
            'bct,cdf->bdtf', x[:, :, :, 1024], row[:, :, lo:hi])
    if np.any(const):
        out += const.astype(np.float32)[None, :, None, :]
    return out


# revision 30
# speedup vs baseline: 1.1008x; 1.1008x over previous
"""BandSplit (gather -> per-band MLP -> scatter-add OLA -> /ola) on 8 TRN2 cores.

Strategy
--------
The whole reference computation is linear in x: fold everything into one
block-banded matrix A of shape (C*F, C*F) so that per (b, t) token
out = A^T vec(x) + const (const == 0 here; added on host regardless).
Data-parallel over the 4096 (b, t) tokens across 8 cores, 512 tokens each,
no cross-core communication.

Key structure:
 - Contraction chunks are 64 f-rows x 2 channels interleaved on the 128
   partitions (p = 2*fl + ci).  Halving the row span shrinks each chunk's
   output window (band support), cutting total matmul columns from ~27.8k
   (128-row, per-channel chunks) to ~19.1k cycles, and both input channels
   ride one matmul.  The lone f=1024 row (and the bias constant) is added
   on host in exact f32, so the device stream is 16 uniform chunks.
 - The packed band matrix `ab` ships as fp8 e3m4 scaled by 2^6 with the
   2^-6 folded into the host-side bf16 cast of x, so PSUM holds true-scale
   outputs and drains are plain f32->f16 copies.  Mixed bf16(x) x fp8(A)
   matmuls run at the full 1 col/cycle rate.
 - Every dma_start blocks its issuing sequencer for ~600 ns (descriptor
   generation for 128 partition lines) and a queue's DMAs move FIFO with
   ~0.5-1 us completion latency each, so DMAs are few and fat (xs in
   chunk-pairs, ab in 2 slabs) and spread over three parallel queues (SP +
   ACT HWDGE rings, GPSIMD SWDGE); y stores bank-major so each PSUM bank
   ships as one contiguous DMA.
 - The stream runs REVERSED (j15 -> j0): wide chunks demand ~2x fewer
   bytes per PE cycle, so consuming them first matches the delivery ramp
   while the narrow, bandwidth-hungry chunks run last from resident data.
 - A 32-matmul junk burst bridges PE-boot -> first-data AND covers a full
   free-running HAM window (3.4 us) so the clock latches 2.4 GHz before
   the real stream; mid-stream stalls under ~3.4 us then cannot de-latch.
 - PSUM bank lifetimes are 2-colored per token chunk (8 banks total);
   each bank drains (DVE/ACT alternating) right after its last chunk and
   stores immediately, with the final bank stored per-tch to overlap the
   postamble edge.
"""

import numpy as np

_P = 128
_C = 2
_F = 1025
_R = 64                     # f-rows per contraction chunk (x2 ci = 128)
_NJ = 16                    # chunks j cover f in [64j, 64j+64); f=1024 is added on host
_TCH = 4                    # token chunks (of 128) per core
_TCORE = _TCH * _P          # 512 tokens per core
_PS_W = _C * _F             # 2050 output columns (col = 2*fo + co)
# PSUM banks: 512-col pieces; the last 2 cols ride a separate 2-col bank but
# share bank 3's slot in the output layout (width 514).
_BANKS = [(0, 512), (512, 1024), (1024, 1536), (1536, 2048), (2048, 2050)]
_LAYW = [512, 512, 512, 514]          # output-layout widths (bank4 merged into 3)
_LAYB = [0, 2048, 4096, 6144]         # block base: b*4*width
_YW = 6144 + 4 * 514                  # 8200
_SCALE_BITS = 6             # ab * 2^6 in fp8; x * 2^-6 in bf16


def _fold_matrix(pre_w, pre_b, post_w, post_b, idx, melw, mask, ola_window):
    """Fold the full reference computation into (A, const).

    A: (C, F, C, F) with out[co, fo] = sum_{ci, fi} x[ci, fi] * A[ci, fi, co, fo]
    const: (C, F) additive constant from the biases.
    """
    K, W = idx.shape
    C = _C
    F = ola_window.shape[0]

    pre_w = np.asarray(pre_w, np.float64)
    post_w = np.asarray(post_w, np.float64)
    pre_b = np.asarray(pre_b, np.float64)
    post_b = np.asarray(post_b, np.float64)
    wts = (np.asarray(melw, np.float64) * np.asarray(mask, np.float64))
    msk = np.asarray(mask, np.float64)
    idx = np.asarray(idx)

    M = np.einsum('kio,koj->kij', pre_w, post_w).reshape(K, W, C, W, C)
    vals = M * wts[:, :, None, None, None] * msk[:, None, None, :, None]

    fin = idx[:, :, None, None, None].astype(np.int64)
    fout = idx[:, None, None, :, None].astype(np.int64)
    cin = np.arange(C)[None, None, :, None, None]
    cout = np.arange(C)[None, None, None, None, :]
    flat = ((cin * F + fin) * C + cout) * F + fout
    A = np.bincount(
        np.broadcast_to(flat, vals.shape).ravel(), weights=vals.ravel(),
        minlength=C * F * C * F,
    ).reshape(C, F, C, F)
    A /= ola_window[None, None, None, :]

    bv = (np.einsum('ko,koj->kj', pre_b, post_w) + post_b).reshape(K, W, C)
    bv = bv * msk[:, :, None]
    cflat = (np.arange(C)[None, None, :] * F + idx[:, :, None]).astype(np.int64)
    const = np.bincount(
        np.broadcast_to(cflat, bv.shape).ravel(), weights=bv.ravel(),
        minlength=C * F,
    ).reshape(C, F)
    const /= ola_window[None, :]
    return A, const


def _plan(A, order=None):
    """Windows, packed offsets, bank touch lists and PSUM slot colors.

    `order` is the stream emission order over chunks (default ascending).
    Bank first/last are STREAM POSITIONS (indices into `order`).
    """
    if order is None:
        order = list(range(_NJ))
    wins = []                   # j -> (lo, hi) in fo units
    for j in range(_NJ):
        f0, f1 = j * _R, min((j + 1) * _R, _F)
        blk = A[:, f0:f1, :, :]
        cols = (blk != 0).any(axis=(0, 1, 2))
        nzc = np.nonzero(cols)[0]
        assert len(nzc) > 0
        wins.append((int(nzc[0]), int(nzc[-1]) + 1))
    covered = np.zeros(_F, bool)
    for lo, hi in wins:
        covered[lo:hi] = True
    assert covered.all(), "window coverage hole"

    offs = {}
    tw = 0
    for j in range(_NJ):
        offs[j] = tw
        tw += (2 * (wins[j][1] - wins[j][0]) + 15) // 16 * 16

    touches = {}                # b -> ordered [(pos, s, e)] in stream order
    for pos, j in enumerate(order):
        lo2, hi2 = 2 * wins[j][0], 2 * wins[j][1]
        for b, (bs, be) in enumerate(_BANKS):
            s, e = max(lo2, bs), min(hi2, be)
            if s < e:
                touches.setdefault(b, []).append((pos, s, e))
    first_p = {b: t[0][0] for b, t in touches.items()}
    last_p = {b: t[-1][0] for b, t in touches.items()}

    colors = {}
    for b in sorted(touches):
        used = {colors[o] for o in colors
                if not (last_p[o] < first_p[b] or last_p[b] < first_p[o])}
        free = [c for c in "AB" if c not in used]
        assert free, f"PSUM slot coloring needs >2 colors at bank {b}"
        colors[b] = free[0]
    return wins, offs, tw, touches, first_p, last_p, colors, order


def _olay(b, tch):
    """Output-layout (base, width) for PSUM bank b, token chunk tch."""
    lb = min(b, 3)
    base = _LAYB[lb] + tch * _LAYW[lb]
    if b == 4:
        base += 512
    return base


_PROGRAM_CACHE = {}


def _build_program(wins, offs, TW, touches, first_p, last_p, colors, order,
                   n_cores):
    import concourse.tile as tile
    import concourse.mybir as mybir
    from concourse import bacc

    f32 = mybir.dt.float32
    bf16 = mybir.dt.bfloat16
    f16 = mybir.dt.float16
    f8e3 = mybir.dt.float8e3
    P = _P
    XCOLS = _NJ * _TCORE         # 8192 cols: j*512 + tch*128 + tok

    nc = bacc.Bacc("TRN2", target_bir_lowering=False, debug=False,
                   num_devices=n_cores)
    xs = nc.dram_tensor("xs", [P, XCOLS], bf16, kind="ExternalInput")
    ab = nc.dram_tensor("ab", [P, TW], f8e3, kind="ExternalInput")
    # y: bank-major f16; block b at _LAYB[b], piece (b, tch) at _olay(b, tch)
    y = nc.dram_tensor("y", [P, _YW], f16, kind="ExternalOutput")

    with tile.TileContext(nc) as tc:
        with (
            tc.tile_pool(name="apool", bufs=1) as apool,
            tc.tile_pool(name="xpool", bufs=1) as xpool,
            tc.tile_pool(name="opool", bufs=1) as opool,
            tc.tile_pool(name="jpool", bufs=1) as jpool,
            tc.tile_pool(name="pspool", bufs=1, space="PSUM") as pspool,
        ):
            abig = apool.tile([P, TW], f8e3, name="abig")
            xbig = xpool.tile([P, XCOLS], bf16, name="xbig")
            junk = jpool.tile([P, P], bf16, name="junk")
            ot = opool.tile([P, _YW], f16, name="ot")

            nc.vector.memset(junk[:], 0.0)

            # Loads: both HWDGE rings issue in parallel (~600 ns per
            # dma_start on the issuing sequencer), and each ring moves its
            # queue in FIFO order.  The first pieces are small so the j0/j1
            # completion semaphores fire early; per-ring order tracks the
            # stream's consumption order and bytes are balanced per ring.
            def xs_load(eng, j0, j1):
                eng.dma_start(xbig[:, j0 * _TCORE:j1 * _TCORE],
                              xs[:, j0 * _TCORE:j1 * _TCORE])

            def ab_load(eng, j0, j1):
                o0 = offs[j0]
                o1 = offs[j1] if j1 < _NJ else TW
                eng.dma_start(abig[:, o0:o1], ab[:, o0:o1])

            # Reversed stream (j15 -> j0): wide chunks first, whose
            # bytes-per-PE-cycle demand is ~2x lower, so early delivery
            # keeps up; narrow chunks run last when all data is resident.
            # Three parallel queues: SP + ACT (HWDGE) + GPSIMD (SWDGE),
            # each queue's FIFO matching the stream's consumption order.
            xs_load(nc.sync, 14, 16)     # j14, j15 (first on SP)
            ab_load(nc.scalar, 10, 16)   # ab tail slab (first on ACT)
            xs_load(nc.gpsimd, 12, 14)   # j12, j13 (first on GPS)
            xs_load(nc.sync, 10, 12)     # j10, j11
            ab_load(nc.scalar, 0, 10)    # ab head slab
            xs_load(nc.gpsimd, 8, 10)    # j8, j9
            xs_load(nc.sync, 6, 8)       # j6, j7
            xs_load(nc.scalar, 4, 6)     # j4, j5
            xs_load(nc.sync, 2, 4)       # j2, j3
            xs_load(nc.scalar, 0, 2)     # j0, j1

            # Warmup burst: bridge PE-boot -> first data, keep the HAM
            # continuous-activity window alive into the real stream.  The
            # warm tile shares the first-touched bank's PSUM slot.
            lo2_0 = 2 * wins[order[0]][0]
            first_bank = min(b for b, (bs, be) in enumerate(_BANKS)
                             if max(lo2_0, bs) < min(2 * wins[order[0]][1], be))
            warm = pspool.tile([P, 512], f32, tag=f"{colors[first_bank]}0",
                               name="warm")
            for _ in range(32):
                nc.tensor.matmul(warm[:, :P], junk[:], junk[:],
                                 start=True, stop=True)

            cur = {}                    # (tch, b) -> PSUM tile
            drain_rr = [0]

            def drain(tch, b):
                base = _olay(b, tch)
                w = _BANKS[b][1] - _BANKS[b][0]
                t = cur.pop((tch, b))
                if drain_rr[0] % 2 == 0:
                    nc.vector.tensor_copy(ot[:, base:base + w], t[:])
                else:
                    nc.scalar.copy(ot[:, base:base + w], t[:])
                drain_rr[0] += 1

            def emit_mms(pos, tch):
                j = order[pos]
                lo2 = 2 * wins[j][0]
                lhsT = xbig[:, j * _TCORE + tch * P:j * _TCORE + (tch + 1) * P]
                o = offs[j]
                for b, (bs, be) in enumerate(_BANKS):
                    s, e = max(lo2, bs), min(2 * wins[j][1], be)
                    if s >= e:
                        continue
                    if (tch, b) not in cur:
                        cur[(tch, b)] = pspool.tile(
                            [P, be - bs], f32, tag=f"{colors[b]}{tch}",
                            name=f"bk{b}_{tch}")
                    nc.tensor.matmul(
                        cur[(tch, b)][:, s - bs:e - bs],
                        lhsT, abig[:, o + s - lo2:o + e - lo2],
                        start=(touches[b][0][0] == pos),
                        stop=(touches[b][-1][0] == pos),
                    )

            # output-layout groups: bank 4 shares bank 3's 514-wide block;
            # a group stores once ALL member banks have drained
            groups = {0: [0], 1: [1], 2: [2], 3: [3, 4]}
            gstore_pos = {g: max(last_p[b] for b in m if b in touches)
                          for g, m in groups.items()}
            last_pos = len(order) - 1
            store_rr = [0]

            # main stream in `order`; emission matches per-queue arrival
            for pos in range(last_pos):
                for tch in range(_TCH):
                    emit_mms(pos, tch)
                for b in sorted(touches):
                    if last_p[b] == pos:
                        for tch in range(_TCH):
                            drain(tch, b)
                for g in groups:
                    if gstore_pos[g] == pos:
                        base, w4 = _LAYB[g], 4 * _LAYW[g]
                        eng = nc.sync if store_rr[0] % 2 == 0 else nc.scalar
                        store_rr[0] += 1
                        eng.dma_start(y[:, base:base + w4],
                                      ot[:, base:base + w4])

            # tail: per token chunk, the last chunk + drains + per-tch
            # quarter stores, alternating rings
            tail_banks = sorted(b for b in touches if last_p[b] == last_pos)
            tail_groups = sorted(g for g in groups if gstore_pos[g] == last_pos)
            for tch in range(_TCH):
                emit_mms(last_pos, tch)
                for b in tail_banks:
                    drain(tch, b)
                for g in tail_groups:
                    base, w = _LAYB[g], _LAYW[g]
                    eng = nc.sync if (tch + g) % 2 == 0 else nc.scalar
                    eng.dma_start(y[:, base + tch * w:base + (tch + 1) * w],
                                  ot[:, base + tch * w:base + (tch + 1) * w])

    nc.compile()
    return nc


def kernel(**inputs):
    import ml_dtypes

    x = np.ascontiguousarray(np.asarray(inputs["x"], np.float32))
    B, C, T, F = x.shape
    assert (B, C, F) == (4, 2, 1025), (B, C, F)
    N_CORES = 8
    TS = T // N_CORES

    A, const = _fold_matrix(
        inputs["pre_w"], inputs["pre_b"], inputs["post_w"], inputs["post_b"],
        inputs["idx"], inputs["melw"], inputs["mask"], inputs["ola_window"],
    )
    A = A.astype(np.float32)
    order = list(range(_NJ - 1, -1, -1))      # reversed stream: j15 -> j0
    wins, offs, TW, touches, first_p, last_p, colors, order = _plan(A, order)

    # packed fp8 band tensor, scaled by 2^6
    ab = np.zeros((_P, TW), ml_dtypes.float8_e3m4)
    for j in range(_NJ):
        lo, hi = wins[j]
        f0, f1 = j * _R, min((j + 1) * _R, _F)
        blk = A[:, f0:f1, :, lo:hi]                   # (ci, fl, co, w)
        q = np.clip(blk * np.float32(2.0 ** _SCALE_BITS), -15.5, 15.5)
        t = q.transpose(1, 0, 3, 2).reshape(2 * (f1 - f0), 2 * (hi - lo))
        ab[0:2 * (f1 - f0), offs[j]:offs[j] + 2 * (hi - lo)] = \
            t.astype(ml_dtypes.float8_e3m4)

    key = (TW, tuple(wins), tuple(order), N_CORES)
    if key not in _PROGRAM_CACHE:
        _PROGRAM_CACHE[key] = _build_program(
            wins, offs, TW, touches, first_p, last_p, colors, order, N_CORES)
    nc = _PROGRAM_CACHE[key]

    # host-side cast: bf16(x * 2^-6); the 2^6 lives in ab
    xq = (x * np.float32(2.0 ** -_SCALE_BITS)).astype(ml_dtypes.bfloat16)

    in_maps = []
    for m in range(N_CORES):
        sl = xq[:, :, m * TS:(m + 1) * TS, :1024]     # (tch, ci, t, 1024)
        sl = sl.reshape(_TCH, _C, TS, _NJ, _R)        # (tch, ci, t, j, fl)
        xs_m = np.ascontiguousarray(
            sl.transpose(4, 1, 3, 0, 2)               # (fl, ci, j, tch, t)
        ).reshape(_P, _NJ * _TCORE)
        in_maps.append({"xs": xs_m, "ab": ab})

    try:
        import antenv.axon_hooks  # noqa: F401
    except ImportError:
        import sys
        import types
        import antenv
        stub = types.ModuleType("antenv.axon_hooks")
        stub.get_axon_ntff_profile_hook = lambda: None
        stub.set_axon_ntff_profile_hook = lambda h: None
        sys.modules["antenv.axon_hooks"] = stub
        antenv.axon_hooks = stub

    from concourse.bass_utils import run_bass_kernel_spmd
    res = run_bass_kernel_spmd(nc, in_maps, core_ids=list(range(N_CORES)))
    globals()["_LAST_RESULT"] = res

    out = np.empty((B, C, T, F), np.float32)
    for m in range(N_CORES):
        ym = res.results[m]["y"].astype(np.float32)   # (128, 8200) bank-major
        cols = np.empty((_P, _TCH, _PS_W), np.float32)
        for b in range(4):
            w = _LAYW[b]
            blk = ym[:, _LAYB[b]:_LAYB[b] + 4 * w].reshape(_P, _TCH, w)
            cols[:, :, 512 * b:512 * b + w] = blk
        ym4 = cols.reshape(_P, _TCH, F, C).transpose(1, 3, 0, 2)
        out[:, :, m * TS:(m + 1) * TS, :] = ym4
    # f = 1024 input row: computed on host, unquantized f32 (exact)
    row = A[:, 1024, :, :]                            # (ci, co, fo)
    nzc = np.nonzero((row != 0).any(axis=(0, 1)))[0]
    if len(nzc):
        lo, hi = int(nzc[0]), int(nzc[-1]) + 1
        out[:, :, :, lo:hi] += np.einsum(
            'bct,cdf->bdtf', x[:, :, :, 1024], row[:, :, lo:hi])
    if np.any(const):
        out += const.astype(np.float32)[None, :, None, :]
    return out


# revision 31
# speedup vs baseline: 1.1529x; 1.0474x over previous
"""BandSplit (gather -> per-band MLP -> scatter-add OLA -> /ola) on 8 TRN2 cores.

Strategy
--------
The whole reference computation is linear in x: fold everything into one
block-banded matrix A of shape (C*F, C*F) so that per (b, t) token
out = A^T vec(x) + const (const == 0 here; added on host regardless).
Data-parallel over the 4096 (b, t) tokens across 8 cores, 512 tokens each,
no cross-core communication.

Key structure:
 - Contraction chunks are 64 f-rows x 2 channels interleaved on the 128
   partitions (p = 2*fl + ci).  Halving the row span shrinks each chunk's
   output window (band support), cutting total matmul columns from ~27.8k
   (128-row, per-channel chunks) to ~19.1k cycles, and both input channels
   ride one matmul.  The lone f=1024 row (and the bias constant) is added
   on host in exact f32, so the device stream is 16 uniform chunks.
 - The packed band matrix `ab` ships as fp8 e3m4 scaled by 2^6 with the
   2^-6 folded into the host-side bf16 cast of x, so PSUM holds true-scale
   outputs and drains are plain f32->f16 copies.  Mixed bf16(x) x fp8(A)
   matmuls run at the full 1 col/cycle rate.
 - Every dma_start blocks its issuing sequencer for ~600 ns (descriptor
   generation for 128 partition lines) and a queue's DMAs move FIFO with
   ~0.5-1 us completion latency each, so DMAs are few and fat (xs in
   chunk-pairs, ab in 2 slabs) and spread over three parallel queues (SP +
   ACT HWDGE rings, GPSIMD SWDGE); y stores bank-major so each PSUM bank
   ships as one contiguous DMA.
 - The stream runs REVERSED (j15 -> j0): wide chunks demand ~2x fewer
   bytes per PE cycle, so consuming them first matches the delivery ramp
   while the narrow, bandwidth-hungry chunks run last from resident data.
 - A 32-matmul junk burst bridges PE-boot -> first-data AND covers a full
   free-running HAM window (3.4 us) so the clock latches 2.4 GHz before
   the real stream; mid-stream stalls under ~3.4 us then cannot de-latch.
 - PSUM bank lifetimes are 2-colored per token chunk (8 banks total);
   each bank drains (DVE/ACT alternating) right after its last chunk and
   stores immediately, with the final bank stored per-tch to overlap the
   postamble edge.
"""

import numpy as np

_P = 128
_C = 2
_F = 1025
_R = 64                     # f-rows per contraction chunk (x2 ci = 128)
_NJ = 16                    # chunks j cover f in [64j, 64j+64); f=1024 is added on host
_TCH = 4                    # token chunks (of 128) per core
_TCORE = _TCH * _P          # 512 tokens per core
_PS_W = _C * _F             # 2050 output columns (col = 2*fo + co)
# PSUM banks: 512-col pieces; the last 2 cols ride a separate 2-col bank but
# share bank 3's slot in the output layout (width 514).
_BANKS = [(0, 512), (512, 1024), (1024, 1536), (1536, 2048), (2048, 2050)]
_LAYW = [512, 512, 512, 514]          # output-layout widths (bank4 merged into 3)
_LAYB = [0, 2048, 4096, 6144]         # block base: b*4*width
_YW = 6144 + 4 * 514                  # 8200
_SCALE_BITS = 6             # ab * 2^6 in fp8; x * 2^-6 in bf16


def _fold_matrix(pre_w, pre_b, post_w, post_b, idx, melw, mask, ola_window):
    """Fold the full reference computation into (A, const).

    A: (C, F, C, F) with out[co, fo] = sum_{ci, fi} x[ci, fi] * A[ci, fi, co, fo]
    const: (C, F) additive constant from the biases.
    """
    K, W = idx.shape
    C = _C
    F = ola_window.shape[0]

    pre_w = np.asarray(pre_w, np.float64)
    post_w = np.asarray(post_w, np.float64)
    pre_b = np.asarray(pre_b, np.float64)
    post_b = np.asarray(post_b, np.float64)
    wts = (np.asarray(melw, np.float64) * np.asarray(mask, np.float64))
    msk = np.asarray(mask, np.float64)
    idx = np.asarray(idx)

    M = np.einsum('kio,koj->kij', pre_w, post_w).reshape(K, W, C, W, C)
    vals = M * wts[:, :, None, None, None] * msk[:, None, None, :, None]

    fin = idx[:, :, None, None, None].astype(np.int64)
    fout = idx[:, None, None, :, None].astype(np.int64)
    cin = np.arange(C)[None, None, :, None, None]
    cout = np.arange(C)[None, None, None, None, :]
    flat = ((cin * F + fin) * C + cout) * F + fout
    A = np.bincount(
        np.broadcast_to(flat, vals.shape).ravel(), weights=vals.ravel(),
        minlength=C * F * C * F,
    ).reshape(C, F, C, F)
    A /= ola_window[None, None, None, :]

    bv = (np.einsum('ko,koj->kj', pre_b, post_w) + post_b).reshape(K, W, C)
    bv = bv * msk[:, :, None]
    cflat = (np.arange(C)[None, None, :] * F + idx[:, :, None]).astype(np.int64)
    const = np.bincount(
        np.broadcast_to(cflat, bv.shape).ravel(), weights=bv.ravel(),
        minlength=C * F,
    ).reshape(C, F)
    const /= ola_window[None, :]
    return A, const


def _plan(A, order=None):
    """Windows, packed offsets, bank touch lists and PSUM slot colors.

    `order` is the stream emission order over chunks (default ascending).
    Bank first/last are STREAM POSITIONS (indices into `order`).
    """
    if order is None:
        order = list(range(_NJ))
    wins = []                   # j -> (lo, hi) in fo units
    for j in range(_NJ):
        f0, f1 = j * _R, min((j + 1) * _R, _F)
        blk = A[:, f0:f1, :, :]
        cols = (blk != 0).any(axis=(0, 1, 2))
        nzc = np.nonzero(cols)[0]
        assert len(nzc) > 0
        wins.append((int(nzc[0]), int(nzc[-1]) + 1))
    covered = np.zeros(_F, bool)
    for lo, hi in wins:
        covered[lo:hi] = True
    assert covered.all(), "window coverage hole"

    offs = {}
    tw = 0
    for j in range(_NJ):
        offs[j] = tw
        tw += (2 * (wins[j][1] - wins[j][0]) + 15) // 16 * 16

    touches = {}                # b -> ordered [(pos, s, e)] in stream order
    for pos, j in enumerate(order):
        lo2, hi2 = 2 * wins[j][0], 2 * wins[j][1]
        for b, (bs, be) in enumerate(_BANKS):
            s, e = max(lo2, bs), min(hi2, be)
            if s < e:
                touches.setdefault(b, []).append((pos, s, e))
    first_p = {b: t[0][0] for b, t in touches.items()}
    last_p = {b: t[-1][0] for b, t in touches.items()}

    colors = {}
    for b in sorted(touches):
        used = {colors[o] for o in colors
                if not (last_p[o] < first_p[b] or last_p[b] < first_p[o])}
        free = [c for c in "AB" if c not in used]
        assert free, f"PSUM slot coloring needs >2 colors at bank {b}"
        colors[b] = free[0]
    return wins, offs, tw, touches, first_p, last_p, colors, order


def _olay(b, tch):
    """Output-layout (base, width) for PSUM bank b, token chunk tch."""
    lb = min(b, 3)
    base = _LAYB[lb] + tch * _LAYW[lb]
    if b == 4:
        base += 512
    return base


_PROGRAM_CACHE = {}


def _build_program(wins, offs, TW, touches, first_p, last_p, colors, order,
                   n_cores):
    import concourse.tile as tile
    import concourse.mybir as mybir
    from concourse import bacc

    f32 = mybir.dt.float32
    bf16 = mybir.dt.bfloat16
    f16 = mybir.dt.float16
    f8e3 = mybir.dt.float8e3
    P = _P
    XCOLS = _NJ * _TCORE         # 8192 cols: j*512 + tch*128 + tok

    nc = bacc.Bacc("TRN2", target_bir_lowering=False, debug=False,
                   num_devices=n_cores)
    xs = nc.dram_tensor("xs", [P, XCOLS], bf16, kind="ExternalInput")
    ab = nc.dram_tensor("ab", [P, TW], f8e3, kind="ExternalInput")
    # y: bank-major f16; block b at _LAYB[b], piece (b, tch) at _olay(b, tch)
    y = nc.dram_tensor("y", [P, _YW], f16, kind="ExternalOutput")

    with tile.TileContext(nc) as tc:
        with (
            tc.tile_pool(name="apool", bufs=1) as apool,
            tc.tile_pool(name="xpool", bufs=1) as xpool,
            tc.tile_pool(name="opool", bufs=1) as opool,
            tc.tile_pool(name="jpool", bufs=1) as jpool,
            tc.tile_pool(name="pspool", bufs=1, space="PSUM") as pspool,
        ):
            abig = apool.tile([P, TW], f8e3, name="abig")
            xbig = xpool.tile([P, XCOLS], bf16, name="xbig")
            junk = jpool.tile([P, P], bf16, name="junk")
            ot = opool.tile([P, _YW], f16, name="ot")

            nc.vector.memset(junk[:], 0.0)

            # Loads: both HWDGE rings issue in parallel (~600 ns per
            # dma_start on the issuing sequencer), and each ring moves its
            # queue in FIFO order.  The first pieces are small so the j0/j1
            # completion semaphores fire early; per-ring order tracks the
            # stream's consumption order and bytes are balanced per ring.
            def xs_load(eng, j0, j1):
                eng.dma_start(xbig[:, j0 * _TCORE:j1 * _TCORE],
                              xs[:, j0 * _TCORE:j1 * _TCORE])

            def ab_load(eng, j0, j1):
                o0 = offs[j0]
                o1 = offs[j1] if j1 < _NJ else TW
                eng.dma_start(abig[:, o0:o1], ab[:, o0:o1])

            # Reversed stream (j15 -> j0): wide chunks first, whose
            # bytes-per-PE-cycle demand is ~2x lower, so early delivery
            # keeps up; narrow chunks run last when all data is resident.
            # Three parallel queues: SP + ACT (HWDGE) + GPSIMD (SWDGE),
            # each queue's FIFO matching the stream's consumption order.
            xs_load(nc.sync, 14, 16)     # j14, j15 (first on SP)
            ab_load(nc.scalar, 10, 16)   # ab tail slab (first on ACT)
            xs_load(nc.gpsimd, 12, 14)   # j12, j13 (first on GPS)
            xs_load(nc.sync, 10, 12)     # j10, j11
            ab_load(nc.scalar, 0, 10)    # ab head slab
            xs_load(nc.gpsimd, 8, 10)    # j8, j9
            xs_load(nc.sync, 6, 8)       # j6, j7
            xs_load(nc.scalar, 4, 6)     # j4, j5
            xs_load(nc.sync, 2, 4)       # j2, j3
            xs_load(nc.scalar, 0, 2)     # j0, j1

            # Warmup burst: bridge PE-boot -> first data, keep the HAM
            # continuous-activity window alive into the real stream.  The
            # warm tile shares the first-touched bank's PSUM slot.
            lo2_0 = 2 * wins[order[0]][0]
            first_bank = min(b for b, (bs, be) in enumerate(_BANKS)
                             if max(lo2_0, bs) < min(2 * wins[order[0]][1], be))
            warm = pspool.tile([P, 512], f32, tag=f"{colors[first_bank]}0",
                               name="warm")
            for _ in range(32):
                nc.tensor.matmul(warm[:, :P], junk[:], junk[:],
                                 start=True, stop=True)

            cur = {}                    # (tch, b) -> PSUM tile
            drain_rr = [0]

            def drain(tch, b):
                base = _olay(b, tch)
                w = _BANKS[b][1] - _BANKS[b][0]
                t = cur.pop((tch, b))
                if drain_rr[0] % 2 == 0:
                    nc.vector.tensor_copy(ot[:, base:base + w], t[:])
                else:
                    nc.scalar.copy(ot[:, base:base + w], t[:])
                drain_rr[0] += 1

            def emit_mms(pos, tch):
                j = order[pos]
                lo2 = 2 * wins[j][0]
                lhsT = xbig[:, j * _TCORE + tch * P:j * _TCORE + (tch + 1) * P]
                o = offs[j]
                for b, (bs, be) in enumerate(_BANKS):
                    s, e = max(lo2, bs), min(2 * wins[j][1], be)
                    if s >= e:
                        continue
                    if (tch, b) not in cur:
                        cur[(tch, b)] = pspool.tile(
                            [P, be - bs], f32, tag=f"{colors[b]}{tch}",
                            name=f"bk{b}_{tch}")
                    nc.tensor.matmul(
                        cur[(tch, b)][:, s - bs:e - bs],
                        lhsT, abig[:, o + s - lo2:o + e - lo2],
                        start=(touches[b][0][0] == pos),
                        stop=(touches[b][-1][0] == pos),
                    )

            # output-layout groups: bank 4 shares bank 3's 514-wide block;
            # a group stores once ALL member banks have drained
            groups = {0: [0], 1: [1], 2: [2], 3: [3, 4]}
            gstore_pos = {g: max(last_p[b] for b in m if b in touches)
                          for g, m in groups.items()}
            last_pos = len(order) - 1
            store_rr = [0]

            # main stream in `order`; emission matches per-queue arrival
            for pos in range(last_pos):
                for tch in range(_TCH):
                    emit_mms(pos, tch)
                for b in sorted(touches):
                    if last_p[b] == pos:
                        for tch in range(_TCH):
                            drain(tch, b)
                for g in groups:
                    if gstore_pos[g] == pos:
                        base, w4 = _LAYB[g], 4 * _LAYW[g]
                        eng = nc.sync if store_rr[0] % 2 == 0 else nc.scalar
                        store_rr[0] += 1
                        eng.dma_start(y[:, base:base + w4],
                                      ot[:, base:base + w4])

            # tail: per token chunk, the last chunk + drains + per-tch
            # quarter stores, alternating rings
            tail_banks = sorted(b for b in touches if last_p[b] == last_pos)
            tail_groups = sorted(g for g in groups if gstore_pos[g] == last_pos)
            for tch in range(_TCH):
                emit_mms(last_pos, tch)
                for b in tail_banks:
                    # split the tail drain across DVE + ACT in parallel
                    base = _olay(b, tch)
                    w = _BANKS[b][1] - _BANKS[b][0]
                    t = cur.pop((tch, b))
                    h = w // 2
                    if h:
                        nc.vector.tensor_copy(ot[:, base:base + h], t[:, :h])
                        nc.scalar.copy(ot[:, base + h:base + w], t[:, h:])
                    else:
                        nc.vector.tensor_copy(ot[:, base:base + w], t[:])
                for g in tail_groups:
                    base, w = _LAYB[g], _LAYW[g]
                    eng = nc.sync if (tch + g) % 2 == 0 else nc.scalar
                    eng.dma_start(y[:, base + tch * w:base + (tch + 1) * w],
                                  ot[:, base + tch * w:base + (tch + 1) * w])

    nc.compile()
    return nc


def kernel(**inputs):
    import ml_dtypes

    x = np.ascontiguousarray(np.asarray(inputs["x"], np.float32))
    B, C, T, F = x.shape
    assert (B, C, F) == (4, 2, 1025), (B, C, F)
    N_CORES = 8
    TS = T // N_CORES

    A, const = _fold_matrix(
        inputs["pre_w"], inputs["pre_b"], inputs["post_w"], inputs["post_b"],
        inputs["idx"], inputs["melw"], inputs["mask"], inputs["ola_window"],
    )
    A = A.astype(np.float32)
    order = list(range(_NJ - 1, -1, -1))      # reversed stream: j15 -> j0
    wins, offs, TW, touches, first_p, last_p, colors, order = _plan(A, order)

    # packed fp8 band tensor, scaled by 2^6
    ab = np.zeros((_P, TW), ml_dtypes.float8_e3m4)
    for j in range(_NJ):
        lo, hi = wins[j]
        f0, f1 = j * _R, min((j + 1) * _R, _F)
        blk = A[:, f0:f1, :, lo:hi]                   # (ci, fl, co, w)
        q = np.clip(blk * np.float32(2.0 ** _SCALE_BITS), -15.5, 15.5)
        t = q.transpose(1, 0, 3, 2).reshape(2 * (f1 - f0), 2 * (hi - lo))
        ab[0:2 * (f1 - f0), offs[j]:offs[j] + 2 * (hi - lo)] = \
            t.astype(ml_dtypes.float8_e3m4)

    key = (TW, tuple(wins), tuple(order), N_CORES)
    if key not in _PROGRAM_CACHE:
        _PROGRAM_CACHE[key] = _build_program(
            wins, offs, TW, touches, first_p, last_p, colors, order, N_CORES)
    nc = _PROGRAM_CACHE[key]

    # host-side cast: bf16(x * 2^-6); the 2^6 lives in ab
    xq = (x * np.float32(2.0 ** -_SCALE_BITS)).astype(ml_dtypes.bfloat16)

    in_maps = []
    for m in range(N_CORES):
        sl = xq[:, :, m * TS:(m + 1) * TS, :1024]     # (tch, ci, t, 1024)
        sl = sl.reshape(_TCH, _C, TS, _NJ, _R)        # (tch, ci, t, j, fl)
        xs_m = np.ascontiguousarray(
            sl.transpose(4, 1, 3, 0, 2)               # (fl, ci, j, tch, t)
        ).reshape(_P, _NJ * _TCORE)
        in_maps.append({"xs": xs_m, "ab": ab})

    try:
        import antenv.axon_hooks  # noqa: F401
    except ImportError:
        import sys
        import types
        import antenv
        stub = types.ModuleType("antenv.axon_hooks")
        stub.get_axon_ntff_profile_hook = lambda: None
        stub.set_axon_ntff_profile_hook = lambda h: None
        sys.modules["antenv.axon_hooks"] = stub
        antenv.axon_hooks = stub

    from concourse.bass_utils import run_bass_kernel_spmd
    res = run_bass_kernel_spmd(nc, in_maps, core_ids=list(range(N_CORES)))
    globals()["_LAST_RESULT"] = res

    out = np.empty((B, C, T, F), np.float32)
    for m in range(N_CORES):
        ym = res.results[m]["y"].astype(np.float32)   # (128, 8200) bank-major
        cols = np.empty((_P, _TCH, _PS_W), np.float32)
        for b in range(4):
            w = _LAYW[b]
            blk = ym[:, _LAYB[b]:_LAYB[b] + 4 * w].reshape(_P, _TCH, w)
            cols[:, :, 512 * b:512 * b + w] = blk
        ym4 = cols.reshape(_P, _TCH, F, C).transpose(1, 3, 0, 2)
        out[:, :, m * TS:(m + 1) * TS, :] = ym4
    # f = 1024 input row: computed on host, unquantized f32 (exact)
    row = A[:, 1024, :, :]                            # (ci, co, fo)
    nzc = np.nonzero((row != 0).any(axis=(0, 1)))[0]
    if len(nzc):
        lo, hi = int(nzc[0]), int(nzc[-1]) + 1
        out[:, :, :, lo:hi] += np.einsum(
            'bct,cdf->bdtf', x[:, :, :, 1024], row[:, :, lo:hi])
    if np.any(const):
        out += const.astype(np.float32)[None, :, None, :]
    return out
